# revision 14
# baseline (speedup 1.0000x reference)
"""Chunked-causal attention (MemoryEfficientAttention) for Trainium2.

Full inputs q,k,v: [2, 16, 2048, 64] fp32. Causal attention per (batch, head);
chunked reference == plain causal attention. 32 (b,h) slices split 4-per-core
across 8 NeuronCores (data/head parallel, no collectives).

Per-core kernel (4 heads, S=2048, D=64), v3 design:
  - q,k loaded natural [128, 16, 64], cast to fp16 into head-PAIR tiles
    [128, 16, 128] (head a in cols 0:64, head b in 64:128), then transposed
    d-major by the DMA XBAR (InstDmaTransposeAnt): its fold semantics
    out[r % 128, r // 128, p] = in[p, r] land head a's qT on partitions 0:64
    and head b's on 64:128 - zero PE cost, no fixup copies. Head b's matmuls
    use base partition 64 (PE tile position (64, 0)).
  - [V|1] cast to fp16 [128, 16, 65] (ones col -> softmax denominator).
  - The two heads of a pair run INTERLEAVED per (pass, key-block) unit so
    every engine always has an independent unit in flight (head a's AV never
    waits on head b's exp):
      scT[j,i] = kT_jb.T @ qT   (fp16 -> fp32 PSUM, <=512 col chunks)
      eT = exp(scT/8) fp16      (one ACT instr per unit)
      diagonal block masked in place on GPSIMD (affine_select, fill 0)
      acc[d|1, i] += vext_jb.T @ eT   (lag-1 behind QK in the PE queue)
  - epilogue per pass: acc -> oT fp16 [80, 8, 128] (DVE; rows 65:80 pad),
    one XBAR transpose -> o_ti [128, 8, 80] i-major, rcp = 1/denom (DVE),
    out = val*rcp -> f32 (DVE), single DMA store per pass.
  - queue split: input loads on the sync DGE; XBARs issued from the DVE DGE
    (naturally ordered after the DVE casts that feed them); output stores on
    the GPSIMD DGE. First QK starts after ~4 loads + 1 cast + 1 XBAR.

PSUM: scT [128,1024] x2 (4 banks) + acc [65,1024] x1 per head (4 banks) = 8.
Softmax without max-subtraction: scores/8 ~ N(0,1), far from fp32 exp range.
"""

import hashlib
import os

import numpy as np

B, H, S, D = 2, 16, 2048, 64
N_CORES = 8
HPC = (B * H) // N_CORES  # heads per core
NB = S // 128             # 128-row key blocks per head
PASS_W = 1024             # query pass width (2 PSUM banks)
CHUNK = 512               # AV accumulator chunk (1 PSUM bank)

_NC = None


def _install_neff_cache():
    """Content-addressed NEFF cache so repeat runs skip the ~2min walrus compile."""
    import concourse.bass2jax as bass2jax

    real_compile = bass2jax.compile_bir_kernel
    if getattr(bass2jax, "_neff_cache_installed", False):
        return
    cache_dir = os.path.expanduser("~/.cache/bass_neff")
    os.makedirs(cache_dir, exist_ok=True)

    def cached_compile(bir_json, tmpdir, neff_name="file.neff"):
        key = hashlib.sha256(bir_json).hexdigest()[:24]
        path = os.path.join(cache_dir, f"{key}.neff")
        if os.path.exists(path):
            dst = os.path.join(tmpdir, neff_name)
            with open(path, "rb") as f_in, open(dst, "wb") as f_out:
                f_out.write(f_in.read())
            return dst
        neff = real_compile(bir_json, tmpdir, neff_name)
        with open(neff, "rb") as f_in, open(path + ".tmp", "wb") as f_out:
            f_out.write(f_in.read())
        os.replace(path + ".tmp", path)
        return neff

    bass2jax.compile_bir_kernel = cached_compile
    bass2jax._neff_cache_installed = True


def _build():
    import concourse.bacc as bacc
    import concourse.mybir as mybir
    import concourse.tile as tile

    f32 = mybir.dt.float32
    f16 = mybir.dt.float16
    Exp = mybir.ActivationFunctionType.Exp

    nc = bacc.Bacc()
    q_d = nc.dram_tensor("q", [HPC, S, D], f32, kind="ExternalInput")
    k_d = nc.dram_tensor("k", [HPC, S, D], f32, kind="ExternalInput")
    v_d = nc.dram_tensor("v", [HPC, S, D], f32, kind="ExternalInput")
    o_d = nc.dram_tensor("out", [HPC, S, D], f32, kind="ExternalOutput")

    with tile.TileContext(nc) as tc:
        with (
            tc.tile_pool(name="sb", bufs=2) as sb,
            tc.tile_pool(name="exps", bufs=6) as exps,
            tc.tile_pool(name="epi", bufs=2) as epi,
            tc.tile_pool(name="ps", bufs=2, space="PSUM") as ps,
        ):
            def stage(pair):
                # ---- load + stage (two heads per XBAR transpose) ------
                q2 = sb.tile([128, NB, 128], f16, name=f"q2{pair}", tag="q2")
                k2 = sb.tile([128, NB, 128], f16, name=f"k2{pair}", tag="k2")
                for i in range(2):
                    h = 2 * pair + i
                    q_nat = sb.tile(
                        [128, NB, D], f32, name=f"qn{h}", tag=f"qn{i}"
                    )
                    k_nat = sb.tile(
                        [128, NB, D], f32, name=f"kn{h}", tag=f"kn{i}"
                    )
                    nc.sync.dma_start(
                        out=q_nat,
                        in_=q_d[h].rearrange("(n p) d -> p n d", p=128),
                    )
                    nc.sync.dma_start(
                        out=k_nat,
                        in_=k_d[h].rearrange("(n p) d -> p n d", p=128),
                    )
                    nc.vector.tensor_copy(q2[:, :, i * D : (i + 1) * D], q_nat)
                    nc.vector.tensor_copy(k2[:, :, i * D : (i + 1) * D], k_nat)
                qT2 = sb.tile([128, S], f16, name=f"qT{pair}", tag="qT")
                kT2 = sb.tile([128, S], f16, name=f"kT{pair}", tag="kT")
                nc.sync.dma_start_transpose(
                    qT2.rearrange("d (n p) -> d n p", p=128), q2
                )
                nc.sync.dma_start_transpose(
                    kT2.rearrange("d (n p) -> d n p", p=128), k2
                )
                vexts = []
                for i in range(2):
                    h = 2 * pair + i
                    v_stg = sb.tile(
                        [128, NB, D + 1], f32, name=f"vs{h}", tag=f"vs{i}"
                    )
                    nc.sync.dma_start(
                        out=v_stg[:, :, 0:D],
                        in_=v_d[h].rearrange("(n p) d -> p n d", p=128),
                    )
                    nc.gpsimd.memset(v_stg[:, :, D], 1.0)
                    vext = sb.tile(
                        [128, NB, D + 1], f16, name=f"vx{h}", tag=f"vx{i}"
                    )
                    nc.vector.tensor_copy(vext, v_stg)
                    vexts.append(vext)
                qT = [qT2[i * D : (i + 1) * D, :] for i in range(2)]
                kT = [kT2[i * D : (i + 1) * D, :] for i in range(2)]
                return qT, kT, vexts

            staged = {0: stage(0)}
            for pair in range(HPC // 2):
                if pair + 1 < HPC // 2:
                    staged[pair + 1] = stage(pair + 1)
                qT, kT, vexts = staged.pop(pair)

                # ---- main loop: both heads interleaved per unit -------
                for p in range(2):
                    ilo_p, ihi_p = p * PASS_W, (p + 1) * PASS_W
                    n_jb = 8 * p + 8
                    accs = [
                        ps.tile(
                            [D + 1, PASS_W],
                            f32,
                            tag=f"acc{i}",
                            bufs=1,
                            name=f"ac{i}{p}",
                        )
                        for i in range(2)
                    ]
                    eTs = {}

                    def emit_qk_exp(i, jb):
                        j0 = jb * 128
                        i_lo = max(j0, ilo_p)
                        w = ihi_p - i_lo
                        scT = ps.tile(
                            [128, PASS_W], f32, tag="sc", bufs=2, name="scT"
                        )
                        for c0 in range(0, w, CHUNK):
                            c1 = min(c0 + CHUNK, w)
                            nc.tensor.matmul(
                                scT[:, c0:c1],
                                kT[i][:, j0 : j0 + 128],
                                qT[i][:, i_lo + c0 : i_lo + c1],
                                start=True,
                                stop=True,
                            )
                        eT = exps.tile([128, PASS_W], f16, tag="eT", name="eT")
                        eTs[(i, jb)] = eT
                        nc.scalar.activation(
                            eT[:, 0:w], scT[:, 0:w], Exp, scale=float(D) ** -0.5
                        )
                        if j0 >= ilo_p:
                            # diagonal block: keep j <= i (iota = i - j >= 0)
                            nc.gpsimd.affine_select(
                                out=eT[:, 0:128],
                                in_=eT[:, 0:128],
                                compare_op=mybir.AluOpType.is_ge,
                                fill=0.0,
                                pattern=[[1, 128]],
                                channel_multiplier=-1,
                            )

                    def emit_av(i, jb):
                        i_lo = max(jb * 128, ilo_p)
                        for c in range(2):
                            g = 2 * p + c
                            ch_lo = ilo_p + c * CHUNK
                            ch_hi = ilo_p + (c + 1) * CHUNK
                            if ch_hi <= i_lo:
                                continue
                            a_lo = max(i_lo, ch_lo)
                            nc.tensor.matmul(
                                accs[i][:, a_lo - ilo_p : ch_hi - ilo_p],
                                vexts[i][:, jb, :],
                                eTs[(i, jb)][:, a_lo - i_lo : ch_hi - i_lo],
                                start=(jb == 0),
                                stop=(jb == 4 * g + 3),
                            )

                    for jb in range(n_jb):
                        for i in range(2):
                            emit_qk_exp(i, jb)
                        if jb >= 1:
                            for i in range(2):
                                emit_av(i, jb - 1)
                    for i in range(2):
                        emit_av(i, n_jb - 1)

                    # ---- epilogue both heads: XBAR transpose + norm ---
                    oTs, otis = [], []
                    for i in range(2):
                        oT = epi.tile([80, 8, 128], f16, tag=f"oT{i}", name="oT")
                        nc.gpsimd.memset(oT[64:80], 0.0)
                        nc.vector.tensor_copy(
                            oT[0 : D + 1],
                            accs[i].rearrange("d (b pp) -> d b pp", pp=128),
                        )
                        o_ti = epi.tile(
                            [128, 8, 80], f16, tag=f"oti{i}", name="oti"
                        )
                        nc.sync.dma_start_transpose(o_ti, oT)
                        oTs.append(oT)
                        otis.append(o_ti)
                    for i in range(2):
                        h = 2 * pair + i
                        o_ti = otis[i]
                        rcp = epi.tile(
                            [128, 8, 1], f32, tag=f"rcp{i}", name="rcp"
                        )
                        nc.vector.reciprocal(rcp, o_ti[:, :, D : D + 1])
                        o_f = epi.tile(
                            [128, 8, D], f32, tag=f"of{i}", name="of"
                        )
                        nc.vector.tensor_mul(
                            o_f, o_ti[:, :, 0:D], rcp.broadcast_to([128, 8, D])
                        )
                        nc.gpsimd.dma_start(
                            out=o_d[h].rearrange("(n p) d -> p n d", p=128)[
                                :, 8 * p : 8 * p + 8, :
                            ],
                            in_=o_f,
                        )

    nc.finalize()
    return nc


def _get_nc():
    global _NC
    if _NC is None:
        _install_neff_cache()
        _NC = _build()
    return _NC


def kernel(q, k, v):
    from concourse.bass_utils import run_bass_kernel_spmd

    nc = _get_nc()
    q = np.asarray(q, dtype=np.float32).reshape(B * H, S, D)
    k = np.asarray(k, dtype=np.float32).reshape(B * H, S, D)
    v = np.asarray(v, dtype=np.float32).reshape(B * H, S, D)
    in_maps = [
        {
            "q": q[c * HPC : (c + 1) * HPC],
            "k": k[c * HPC : (c + 1) * HPC],
            "v": v[c * HPC : (c + 1) * HPC],
        }
        for c in range(N_CORES)
    ]
    res = run_bass_kernel_spmd(nc, in_maps, core_ids=list(range(N_CORES)))
    out = np.stack([res.results[c]["out"] for c in range(N_CORES)])
    return out.reshape(B, H, S, D).astype(np.float32)


# revision 19
# speedup vs baseline: 1.0128x; 1.0128x over previous
"""Chunked-causal attention (MemoryEfficientAttention) for Trainium2.

Full inputs q,k,v: [2, 16, 2048, 64] fp32. Causal attention per (batch, head);
chunked reference == plain causal attention. 32 (b,h) slices split 4-per-core
across 8 NeuronCores (data/head parallel, no collectives).

Per-core kernel (4 heads, S=2048, D=64), v3 design:
  - q,k loaded natural [128, 16, 64], cast to fp16 into head-PAIR tiles
    [128, 16, 128] (head a in cols 0:64, head b in 64:128), then transposed
    d-major by the DMA XBAR (InstDmaTransposeAnt): its fold semantics
    out[r % 128, r // 128, p] = in[p, r] land head a's qT on partitions 0:64
    and head b's on 64:128 - zero PE cost, no fixup copies. Head b's matmuls
    use base partition 64 (PE tile position (64, 0)).
  - [V|1] cast to fp16 [128, 16, 65] (ones col -> softmax denominator).
  - The two heads of a pair run INTERLEAVED per (pass, key-block) unit so
    every engine always has an independent unit in flight (head a's AV never
    waits on head b's exp):
      scT[j,i] = kT_jb.T @ qT   (fp16 -> fp32 PSUM, <=512 col chunks)
      eT = exp(scT/8) fp16      (one ACT instr per unit)
      diagonal block masked in place on GPSIMD (affine_select, fill 0)
      acc[d|1, i] += vext_jb.T @ eT   (lag-1 behind QK in the PE queue)
  - epilogue per pass: acc -> oT fp16 [80, 8, 128] (DVE; rows 65:80 pad),
    one XBAR transpose -> o_ti [128, 8, 80] i-major, rcp = 1/denom (DVE),
    out = val*rcp -> f32 (DVE), single DMA store per pass.
  - queue split: input loads on the sync DGE; XBARs issued from the DVE DGE
    (naturally ordered after the DVE casts that feed them); output stores on
    the GPSIMD DGE. First QK starts after ~4 loads + 1 cast + 1 XBAR.

PSUM: scT [128,1024] x2 (4 banks) + acc [65,1024] x1 per head (4 banks) = 8.
Softmax without max-subtraction: scores/8 ~ N(0,1), far from fp32 exp range.
"""

import hashlib
import os

import numpy as np

B, H, S, D = 2, 16, 2048, 64
N_CORES = 8
HPC = (B * H) // N_CORES  # heads per core
NB = S // 128             # 128-row key blocks per head
PASS_W = 1024             # query pass width (2 PSUM banks)
CHUNK = 512               # AV accumulator chunk (1 PSUM bank)

_NC = None


def _install_neff_cache():
    """Content-addressed NEFF cache so repeat runs skip the ~2min walrus compile."""
    import concourse.bass2jax as bass2jax

    real_compile = bass2jax.compile_bir_kernel
    if getattr(bass2jax, "_neff_cache_installed", False):
        return
    cache_dir = os.path.expanduser("~/.cache/bass_neff")
    os.makedirs(cache_dir, exist_ok=True)

    def cached_compile(bir_json, tmpdir, neff_name="file.neff"):
        key = hashlib.sha256(bir_json).hexdigest()[:24]
        path = os.path.join(cache_dir, f"{key}.neff")
        if os.path.exists(path):
            dst = os.path.join(tmpdir, neff_name)
            with open(path, "rb") as f_in, open(dst, "wb") as f_out:
                f_out.write(f_in.read())
            return dst
        neff = real_compile(bir_json, tmpdir, neff_name)
        with open(neff, "rb") as f_in, open(path + ".tmp", "wb") as f_out:
            f_out.write(f_in.read())
        os.replace(path + ".tmp", path)
        return neff

    bass2jax.compile_bir_kernel = cached_compile
    bass2jax._neff_cache_installed = True


def _build():
    import concourse.bacc as bacc
    import concourse.mybir as mybir
    import concourse.tile as tile

    f32 = mybir.dt.float32
    f16 = mybir.dt.float16
    Exp = mybir.ActivationFunctionType.Exp

    nc = bacc.Bacc()
    q_d = nc.dram_tensor("q", [HPC, S, D], f32, kind="ExternalInput")
    k_d = nc.dram_tensor("k", [HPC, S, D], f32, kind="ExternalInput")
    v_d = nc.dram_tensor("v", [HPC, S, D], f32, kind="ExternalInput")
    o_d = nc.dram_tensor("out", [HPC, S, D], f32, kind="ExternalOutput")

    with tile.TileContext(nc) as tc:
        with (
            tc.tile_pool(name="sb", bufs=2) as sb,
            tc.tile_pool(name="exps", bufs=6) as exps,
            tc.tile_pool(name="epi", bufs=2) as epi,
            tc.tile_pool(name="ps", bufs=2, space="PSUM") as ps,
        ):
            def stage(pair):
                # ---- load + stage, one XBAR per head-tensor -----------
                # qT/kT are [128, S] per head with rows 64:128 ZERO, so the
                # QK matmuls contract over K=128 like the AV matmuls - the
                # PE tile config never changes (a tile-size switch flushes
                # the array and costs ~2.5x throughput).
                qT, kT, vexts = [], [], []
                for i in range(2):
                    h = 2 * pair + i
                    q_nat = sb.tile(
                        [128, NB, D], f32, name=f"qn{h}", tag=f"qn{i}"
                    )
                    k_nat = sb.tile(
                        [128, NB, D], f32, name=f"kn{h}", tag=f"kn{i}"
                    )
                    nc.sync.dma_start(
                        out=q_nat,
                        in_=q_d[h].rearrange("(n p) d -> p n d", p=128),
                    )
                    nc.sync.dma_start(
                        out=k_nat,
                        in_=k_d[h].rearrange("(n p) d -> p n d", p=128),
                    )
                    q2 = sb.tile([128, NB, 128], f16, name=f"q2{h}", tag=f"q2{i}")
                    k2 = sb.tile([128, NB, 128], f16, name=f"k2{h}", tag=f"k2{i}")
                    nc.gpsimd.memset(q2[:, :, D:128], 0.0)
                    nc.gpsimd.memset(k2[:, :, D:128], 0.0)
                    nc.vector.tensor_copy(q2[:, :, 0:D], q_nat)
                    nc.vector.tensor_copy(k2[:, :, 0:D], k_nat)
                    qTh = sb.tile([128, S], f16, name=f"qT{h}", tag=f"qT{i}")
                    kTh = sb.tile([128, S], f16, name=f"kT{h}", tag=f"kT{i}")
                    nc.sync.dma_start_transpose(
                        qTh.rearrange("d (n p) -> d n p", p=128), q2
                    )
                    nc.sync.dma_start_transpose(
                        kTh.rearrange("d (n p) -> d n p", p=128), k2
                    )
                    qT.append(qTh)
                    kT.append(kTh)
                for i in range(2):
                    h = 2 * pair + i
                    v_stg = sb.tile(
                        [128, NB, D + 1], f32, name=f"vs{h}", tag=f"vs{i}"
                    )
                    nc.sync.dma_start(
                        out=v_stg[:, :, 0:D],
                        in_=v_d[h].rearrange("(n p) d -> p n d", p=128),
                    )
                    nc.gpsimd.memset(v_stg[:, :, D], 1.0)
                    vext = sb.tile(
                        [128, NB, D + 1], f16, name=f"vx{h}", tag=f"vx{i}"
                    )
                    nc.vector.tensor_copy(vext, v_stg)
                    vexts.append(vext)
                return qT, kT, vexts

            staged = {0: stage(0)}
            for pair in range(HPC // 2):
                if pair + 1 < HPC // 2:
                    staged[pair + 1] = stage(pair + 1)
                qT, kT, vexts = staged.pop(pair)

                # ---- main loop: both heads interleaved per unit -------
                # All matmuls contract over K=128 (QK via zero-padded rows)
                # so the PE tile config never changes.
                for p in range(2):
                    ilo_p, ihi_p = p * PASS_W, (p + 1) * PASS_W
                    n_jb = 8 * p + 8
                    accs = [
                        ps.tile(
                            [D + 1, PASS_W],
                            f32,
                            tag=f"acc{i}",
                            bufs=1,
                            name=f"ac{i}{p}",
                        )
                        for i in range(2)
                    ]
                    eTs = {}

                    def emit_qk(i, jb):
                        j0 = jb * 128
                        i_lo = max(j0, ilo_p)
                        w = ihi_p - i_lo
                        scT = ps.tile(
                            [128, PASS_W], f32, tag="sc", bufs=2, name="scT"
                        )
                        eTs[(i, jb)] = (scT, w, i_lo)
                        for c0 in range(0, w, CHUNK):
                            c1 = min(c0 + CHUNK, w)
                            nc.tensor.matmul(
                                scT[:, c0:c1],
                                kT[i][:, j0 : j0 + 128],
                                qT[i][:, i_lo + c0 : i_lo + c1],
                                start=True,
                                stop=True,
                            )

                    def emit_exp(i, jb):
                        scT, w, _ = eTs[(i, jb)]
                        eT = exps.tile([128, PASS_W], f16, tag="eT", name="eT")
                        nc.scalar.activation(
                            eT[:, 0:w], scT[:, 0:w], Exp, scale=float(D) ** -0.5
                        )
                        if jb * 128 >= ilo_p:
                            # diagonal block: keep j <= i (iota = i - j >= 0)
                            nc.gpsimd.affine_select(
                                out=eT[:, 0:128],
                                in_=eT[:, 0:128],
                                compare_op=mybir.AluOpType.is_ge,
                                fill=0.0,
                                pattern=[[1, 128]],
                                channel_multiplier=-1,
                            )
                        eTs[(i, jb)] = (eT, w, eTs[(i, jb)][2])

                    def emit_av(i, jb):
                        eT, w, i_lo = eTs[(i, jb)]
                        for c in range(2):
                            g = 2 * p + c
                            ch_lo = ilo_p + c * CHUNK
                            ch_hi = ilo_p + (c + 1) * CHUNK
                            if ch_hi <= i_lo:
                                continue
                            a_lo = max(i_lo, ch_lo)
                            nc.tensor.matmul(
                                accs[i][:, a_lo - ilo_p : ch_hi - ilo_p],
                                vexts[i][:, jb, :],
                                eT[:, a_lo - i_lo : ch_hi - i_lo],
                                start=(jb == 0),
                                stop=(jb == 4 * g + 3),
                            )

                    for jb in range(n_jb):
                        for i in range(2):
                            emit_qk(i, jb)
                        for i in range(2):
                            emit_exp(i, jb)
                        if jb >= 1:
                            for i in range(2):
                                emit_av(i, jb - 1)
                    for i in range(2):
                        emit_av(i, n_jb - 1)

                    # ---- epilogue both heads: XBAR transpose + norm ---
                    oTs, otis = [], []
                    for i in range(2):
                        oT = epi.tile([80, 8, 128], f16, tag=f"oT{i}", name="oT")
                        nc.gpsimd.memset(oT[64:80], 0.0)
                        nc.vector.tensor_copy(
                            oT[0 : D + 1],
                            accs[i].rearrange("d (b pp) -> d b pp", pp=128),
                        )
                        o_ti = epi.tile(
                            [128, 8, 80], f16, tag=f"oti{i}", name="oti"
                        )
                        nc.sync.dma_start_transpose(o_ti, oT)
                        oTs.append(oT)
                        otis.append(o_ti)
                    for i in range(2):
                        h = 2 * pair + i
                        o_ti = otis[i]
                        rcp = epi.tile(
                            [128, 8, 1], f32, tag=f"rcp{i}", name="rcp"
                        )
                        nc.vector.reciprocal(rcp, o_ti[:, :, D : D + 1])
                        o_f = epi.tile(
                            [128, 8, D], f32, tag=f"of{i}", name="of"
                        )
                        nc.vector.tensor_mul(
                            o_f, o_ti[:, :, 0:D], rcp.broadcast_to([128, 8, D])
                        )
                        nc.gpsimd.dma_start(
                            out=o_d[h].rearrange("(n p) d -> p n d", p=128)[
                                :, 8 * p : 8 * p + 8, :
                            ],
                            in_=o_f,
                        )

    nc.finalize()
    return nc


def _get_nc():
    global _NC
    if _NC is None:
        _install_neff_cache()
        _NC = _build()
    return _NC


def kernel(q, k, v):
    from concourse.bass_utils import run_bass_kernel_spmd

    nc = _get_nc()
    q = np.asarray(q, dtype=np.float32).reshape(B * H, S, D)
    k = np.asarray(k, dtype=np.float32).reshape(B * H, S, D)
    v = np.asarray(v, dtype=np.float32).reshape(B * H, S, D)
    in_maps = [
        {
            "q": q[c * HPC : (c + 1) * HPC],
            "k": k[c * HPC : (c + 1) * HPC],
            "v": v[c * HPC : (c + 1) * HPC],
        }
        for c in range(N_CORES)
    ]
    res = run_bass_kernel_spmd(nc, in_maps, core_ids=list(range(N_CORES)))
    out = np.stack([res.results[c]["out"] for c in range(N_CORES)])
    return out.reshape(B, H, S, D).astype(np.float32)
